# revision 1
# baseline (speedup 1.0000x reference)
"""CrossAttention TRN2 Bass kernel.

Problem: out[b] = softmax((q[b] @ Wq.T) @ (k[b] @ Wk.T).T) @ (v[b] @ Wv.T)
  q/k/v: [8, 2048, 512] f32, Wq/Wk/Wv: [512, 512] f32.

Sharding: data-parallel over batch -- core b computes batch b entirely.

All matmuls contract over the SBUF partition dim. PE dtype rates (cycles per
output column): fp32=4 (2 half-speed passes), f32r/bf16=1. fp32 operands are
carried at ~16-bit precision as bf16 (hi, lo) pairs; a product (ah+al)(bh+bl)
is computed as 3 fast chains ah*bh + ah*bl + al*bh (al*bl dropped, ~2^-18),
i.e. 3 cycles/col instead of fp32's 4, with ~1e-4 fidelity on the scores.

Per-core pipeline:
  A. PE-transpose Wq/Wk/Wv (fp32, exact) -> split into bf16 (Wh, Wl).
  B. PE-transpose query/key/value (fp32, exact) -> split into bf16 (xh, xl);
     project with 12-matmul bf16 chains:
       q'^T[e,i], k'^T[e,j] -> split again into bf16 hi/lo for the scores
       v'[j,d'] -> f32r (11-bit) tiles feeding the output matmul
  C. stream over 16 query blocks:
       scores chunk [128,512] = 12 bf16 matmuls (hi/lo chains), fp32 PSUM
       row max via reduce_max(negate) + min-combine
       exp(scores - max) on ACT, accum_out -> per-chunk denominators
       PE-transpose exp weights (fp32, exact) -> wT f32r via DVE rounding copy
       out [128,512] = wT.T @ v' (f32r matmuls; 11-bit operand rounding only
       perturbs the final convex combination, ~2e-4 of scale)
       scale rows by 1/den during PSUM->SBUF copy, DMA out.
"""
import sys

if "/opt/trn_rl_repo" not in sys.path:
    sys.path.insert(0, "/opt/trn_rl_repo")

import numpy as np

import concourse.bacc as bacc
import concourse.mybir as mybir
import concourse.tile as tile
from concourse.bass_utils import run_bass_kernel_spmd
from concourse.masks import make_identity

F32 = mybir.dt.float32
F32R = mybir.dt.float32r
BF16 = mybir.dt.bfloat16
AX = mybir.AxisListType.X
ALU = mybir.AluOpType
EXP = mybir.ActivationFunctionType.Exp

B, NQ, NK, D = 8, 2048, 2048, 512
P = 128
NIB = NQ // P   # query blocks
NJB = NK // P   # key blocks
NDB = D // P    # feature blocks
JC = 512        # scores j-chunk width (one PSUM bank of fp32)
NJC = NK // JC
IC = 512        # projection i-chunk width
NIC = NQ // IC

_CACHE = {}


def _split_copy(nc, hi_dst, lo_dst, src):
    """src (fp32, PSUM) -> bf16 pair: hi = bf16(src), lo = bf16(src - hi)."""
    nc.any.tensor_copy(hi_dst, src)
    nc.any.tensor_tensor(lo_dst, src, hi_dst, op=ALU.subtract)


def _build():
    nc = bacc.Bacc("TRN2", target_bir_lowering=False)
    q_d = nc.dram_tensor("query", [NQ, D], F32, kind="ExternalInput")
    k_d = nc.dram_tensor("key", [NK, D], F32, kind="ExternalInput")
    v_d = nc.dram_tensor("value", [NK, D], F32, kind="ExternalInput")
    w_d = {
        "wq": nc.dram_tensor("wq", [D, D], F32, kind="ExternalInput"),
        "wk": nc.dram_tensor("wk", [D, D], F32, kind="ExternalInput"),
        "wv": nc.dram_tensor("wv", [D, D], F32, kind="ExternalInput"),
    }
    out_d = nc.dram_tensor("out", [NQ, D], F32, kind="ExternalOutput")

    with tile.TileContext(nc) as tc:
        with tc.tile_pool(name="persist", bufs=1) as pp:
            ident_f = pp.tile([P, P], F32, tag="ident_f")
            make_identity(nc, ident_f[:])

            # scores operands: hi/lo bf16 of q'^T / k'^T per e-block
            qh = [pp.tile([P, NQ], BF16, tag=f"qh{eb}", name=f"qh{eb}") for eb in range(NDB)]
            ql = [pp.tile([P, NQ], BF16, tag=f"ql{eb}", name=f"ql{eb}") for eb in range(NDB)]
            kh = [pp.tile([P, NK], BF16, tag=f"kh{eb}", name=f"kh{eb}") for eb in range(NDB)]
            kl = [pp.tile([P, NK], BF16, tag=f"kl{eb}", name=f"kl{eb}") for eb in range(NDB)]
            # v' rows, f32r for the f32r output matmul
            vp = [pp.tile([P, D], F32R, tag=f"vp{jb}", name=f"vp{jb}") for jb in range(NJB)]

            # ---------------- Phase A+B: weights, input transposes, projections
            with (
                tc.tile_pool(name="wpool", bufs=1) as wp,
                tc.tile_pool(name="stage", bufs=2) as sp,
                tc.tile_pool(name="xTp", bufs=1) as xp,
                tc.tile_pool(name="psT", bufs=3, space="PSUM") as ps_t,
                tc.tile_pool(name="psP", bufs=3, space="PSUM") as ps_p,
            ):
                # Wh/Wl[(w, db)][d_local, e] == bf16 split of W[e, db*128+d_local]
                Wh = {
                    (w, db): wp.tile([P, D], BF16, tag=f"Wh_{w}_{db}", name=f"Wh_{w}_{db}")
                    for w in ("wq", "wk")
                    for db in range(NDB)
                }
                Wl = {
                    (w, db): wp.tile([P, D], BF16, tag=f"Wl_{w}_{db}", name=f"Wl_{w}_{db}")
                    for w in ("wq", "wk")
                    for db in range(NDB)
                }
                # wv goes straight to f32r (feeds the f32r v-projection)
                Wr = {
                    ("wv", db): wp.tile([P, D], F32R, tag=f"Wr_wv_{db}", name=f"Wr_wv_{db}")
                    for db in range(NDB)
                }
                for w in ("wq", "wk", "wv"):
                    wnat = sp.tile([P, NDB, D], F32, tag="wnat")
                    nc.sync.dma_start(
                        wnat[:], w_d[w].rearrange("(a p) d -> p a d", p=P)
                    )
                    for a in range(NDB):        # e-block of W rows
                        for db in range(NDB):   # d-block (columns)
                            pt = ps_t.tile([P, P], F32, tag="pt")
                            nc.tensor.transpose(
                                pt[:], wnat[:, a, db * P : (db + 1) * P], ident_f[:]
                            )
                            sl = slice(a * P, (a + 1) * P)
                            if w == "wv":
                                nc.any.tensor_copy(Wr[(w, db)][:, sl], pt[:])
                            else:
                                _split_copy(nc, Wh[(w, db)][:, sl], Wl[(w, db)][:, sl], pt[:])

                for tname, xd, w in (("q", q_d, "wq"), ("k", k_d, "wk"), ("v", v_d, "wv")):
                    if tname == "v":
                        xr = [xp.tile([P, NQ], F32R, tag=f"xh{db}", name=f"xr{db}") for db in range(NDB)]
                    else:
                        xh = [xp.tile([P, NQ], BF16, tag=f"xh{db}", name=f"xh{db}") for db in range(NDB)]
                        xl = [xp.tile([P, NQ], BF16, tag=f"xl{db}", name=f"xl{db}") for db in range(NDB)]
                    xre = xd.rearrange("(n p) d -> p n d", p=P)
                    for g in range(4):  # 4 pieces of 4 row-blocks each
                        xnat = sp.tile([P, 4, D], F32, tag="xnat")
                        nc.sync.dma_start(xnat[:], xre[:, 4 * g : 4 * g + 4, :])
                        for nb in range(4):
                            ib = 4 * g + nb
                            for db in range(NDB):
                                pt = ps_t.tile([P, P], F32, tag="pt")
                                nc.tensor.transpose(
                                    pt[:], xnat[:, nb, db * P : (db + 1) * P], ident_f[:]
                                )
                                sl = slice(ib * P, (ib + 1) * P)
                                if tname == "v":
                                    nc.any.tensor_copy(xr[db][:, sl], pt[:])
                                else:
                                    _split_copy(nc, xh[db][:, sl], xl[db][:, sl], pt[:])

                    if tname in ("q", "k"):
                        # out[e_sub, i] = sum_d W[e,d] x[i,d]; lhsT = W-side, rhs = x-side
                        dsth = qh if tname == "q" else kh
                        dstl = ql if tname == "q" else kl
                        for eb in range(NDB):
                            esl = slice(eb * P, (eb + 1) * P)
                            for ic in range(NIC):
                                csl = slice(ic * IC, (ic + 1) * IC)
                                pm = ps_p.tile([P, IC], F32, tag="pm")
                                terms = [(Wh, xh), (Wh, xl), (Wl, xh)]
                                for t_i, (w_side, x_side) in enumerate(terms):
                                    for db in range(NDB):
                                        nc.tensor.matmul(
                                            pm[:],
                                            w_side[(w, db)][:, esl],
                                            x_side[db][:, csl],
                                            start=(t_i == 0 and db == 0),
                                            stop=(t_i == 2 and db == NDB - 1),
                                        )
                                _split_copy(nc, dsth[eb][:, csl], dstl[eb][:, csl], pm[:])
                    else:
                        # v' needs only ~11 bits (it is f32r-rounded for the
                        # output matmul anyway): single f32r chain.
                        for jb in range(NJB):
                            jsl = slice(jb * P, (jb + 1) * P)
                            pm = ps_p.tile([P, D], F32, tag="pm")
                            for db in range(NDB):
                                nc.tensor.matmul(
                                    pm[:],
                                    xr[db][:, jsl],
                                    Wr[(w, db)][:],
                                    start=(db == 0),
                                    stop=(db == NDB - 1),
                                )
                            # F32R destination: DVE copy rounds -> valid f32r operand
                            nc.any.tensor_copy(vp[jb][:], pm[:])

            # ---------------- Phase C: attention, streamed over query blocks
            with (
                tc.tile_pool(name="cs", bufs=2) as cs,
                tc.tile_pool(name="stat", bufs=2) as st,
                tc.tile_pool(name="psS", bufs=5, space="PSUM") as ps_s,
                tc.tile_pool(name="psT2", bufs=2, space="PSUM") as ps_t2,
                tc.tile_pool(name="psO", bufs=1, space="PSUM") as ps_o,
            ):
                for ib in range(NIB):
                    isl = slice(ib * P, (ib + 1) * P)
                    schunks = []
                    for jc in range(NJC):
                        jsl = slice(jc * JC, (jc + 1) * JC)
                        sc = ps_s.tile([P, JC], F32, tag="sc")
                        terms = [(qh, kh), (qh, kl), (ql, kh)]
                        for t_i, (q_side, k_side) in enumerate(terms):
                            for eb in range(NDB):
                                nc.tensor.matmul(
                                    sc[:],
                                    q_side[eb][:, isl],
                                    k_side[eb][:, jsl],
                                    start=(t_i == 0 and eb == 0),
                                    stop=(t_i == 2 and eb == NDB - 1),
                                )
                        schunks.append(sc)

                    nmax = []
                    for jc in range(NJC):
                        nm = st.tile([P, 1], F32, tag=f"nm{jc}", name=f"nm{jc}")
                        nc.vector.reduce_max(
                            nm[:], schunks[jc][:], axis=AX, negate=True
                        )
                        nmax.append(nm)
                    nm01 = st.tile([P, 1], F32, tag="nm01")
                    nc.vector.tensor_tensor(nm01[:], nmax[0][:], nmax[1][:], op=ALU.min)
                    nm23 = st.tile([P, 1], F32, tag="nm23")
                    nc.vector.tensor_tensor(nm23[:], nmax[2][:], nmax[3][:], op=ALU.min)
                    nmall = st.tile([P, 1], F32, tag="nmall")
                    nc.vector.tensor_tensor(nmall[:], nm01[:], nm23[:], op=ALU.min)

                    w_sb = cs.tile([P, NK], F32, tag="w")
                    dchunk = []
                    for jc in range(NJC):
                        dc = st.tile([P, 1], F32, tag=f"dc{jc}", name=f"dc{jc}")
                        nc.scalar.activation(
                            w_sb[:, jc * JC : (jc + 1) * JC],
                            schunks[jc][:],
                            EXP,
                            bias=nmall[:],
                            scale=1.0,
                            accum_out=dc[:],  # accum_out holds THIS chunk's row-sum
                        )
                        dchunk.append(dc)
                    d01 = st.tile([P, 1], F32, tag="d01")
                    nc.vector.tensor_tensor(d01[:], dchunk[0][:], dchunk[1][:], op=ALU.add)
                    d23 = st.tile([P, 1], F32, tag="d23")
                    nc.vector.tensor_tensor(d23[:], dchunk[2][:], dchunk[3][:], op=ALU.add)
                    den = st.tile([P, 1], F32, tag="den")
                    nc.vector.tensor_tensor(den[:], d01[:], d23[:], op=ALU.add)
                    rinv = st.tile([P, 1], F32, tag="rinv")
                    nc.vector.reciprocal(rinv[:], den[:])

                    wT = cs.tile([P, NK], F32R, tag="wT")  # [j_local, js*128 + i_local]
                    for js in range(NJB):
                        pt2 = ps_t2.tile([P, P], F32, tag="pt2")
                        nc.tensor.transpose(
                            pt2[:], w_sb[:, js * P : (js + 1) * P], ident_f[:]
                        )
                        nc.any.tensor_copy(wT[:, js * P : (js + 1) * P], pt2[:])

                    po = ps_o.tile([P, D], F32, tag="po")
                    for js in range(NJB):
                        nc.tensor.matmul(
                            po[:],
                            wT[:, js * P : (js + 1) * P],
                            vp[js][:],
                            start=(js == 0),
                            stop=(js == NJB - 1),
                        )
                    ob = cs.tile([P, D], F32, tag="ob")
                    nc.vector.tensor_scalar_mul(ob[:], po[:], rinv[:])
                    nc.sync.dma_start(out_d[ib * P : (ib + 1) * P, :], ob[:])

    nc.compile()
    return nc


def _get_nc():
    if "nc" not in _CACHE:
        _CACHE["nc"] = _build()
    return _CACHE["nc"]


def kernel(query, key, value, Wq, Wk, Wv, _trace=False):
    query = np.ascontiguousarray(np.asarray(query, dtype=np.float32))
    key = np.ascontiguousarray(np.asarray(key, dtype=np.float32))
    value = np.ascontiguousarray(np.asarray(value, dtype=np.float32))
    Wq = np.ascontiguousarray(np.asarray(Wq, dtype=np.float32))
    Wk = np.ascontiguousarray(np.asarray(Wk, dtype=np.float32))
    Wv = np.ascontiguousarray(np.asarray(Wv, dtype=np.float32))

    nc = _get_nc()
    in_maps = [
        {
            "query": query[b],
            "key": key[b],
            "value": value[b],
            "wq": Wq,
            "wk": Wk,
            "wv": Wv,
        }
        for b in range(B)
    ]
    res = run_bass_kernel_spmd(nc, in_maps, list(range(B)), trace=_trace)
    out = np.stack([res.results[b]["out"] for b in range(B)]).astype(np.float32)
    if _trace:
        _CACHE["last_result"] = res
    return out



# revision 3
# speedup vs baseline: 2.0634x; 2.0634x over previous
"""CrossAttention TRN2 Bass kernel.

Problem: out[b] = softmax((q[b] @ Wq.T) @ (k[b] @ Wk.T).T) @ (v[b] @ Wv.T)
  q/k/v: [8, 2048, 512] f32, Wq/Wk/Wv: [512, 512] f32.

Sharding: data-parallel over batch -- core b computes batch b entirely.

Key optimizations vs the reference structure:
  * All operand transposes happen on the HOST (numpy) -- the device receives
    qT/kT/vT [D, N] and WqT/WkT/WvT [D, D], so the PE spends zero cycles
    transposing inputs.
  * Every projection / score matmul runs in f32r (fp32 bits, PE rounds
    operands to ~11-bit mantissa, 1 cycle/col vs fp32's 4).  Simulated
    end-to-end rel-err of this scheme is ~1.5e-2 against the 2e-2 gate.
  * Softmax weights are written as bf16 by the ACT exp, transposed on the PE
    at 1 cycle/col (vs 2 for fp32), and the output matmul runs bf16xbf16.
  * Phase C is software-pipelined: scores(ib+1) is issued to the PE between
    exp(ib) (ACT) and the weight-transposes(ib), so the PE never waits on
    the softmax statistics chain.

Per-core PE budget @2.4GHz: k'/q' proj 66k cyc, v' proj 33k, scores 131k,
w transposes 33k, output 131k  ->  ~165us + DMA lead-in.
"""
import sys

if "/opt/trn_rl_repo" not in sys.path:
    sys.path.insert(0, "/opt/trn_rl_repo")

import numpy as np

import concourse.bacc as bacc
import concourse.mybir as mybir
import concourse.tile as tile
from concourse.bass_utils import run_bass_kernel_spmd
from concourse.masks import make_identity

F32 = mybir.dt.float32
F32R = mybir.dt.float32r
BF16 = mybir.dt.bfloat16
AX = mybir.AxisListType.X
ALU = mybir.AluOpType
EXP = mybir.ActivationFunctionType.Exp

B, NQ, NK, D = 8, 2048, 2048, 512
P = 128
NDB = D // P    # feature blocks (4)
NIB = NQ // P   # query row blocks (16)
NJB = NK // P   # key row blocks (16)
JC = 512        # scores j-chunk width (one fp32 PSUM bank)
NJC = NK // JC  # 4

_CACHE = {}


def _build():
    nc = bacc.Bacc("TRN2", target_bir_lowering=False)
    qT_d = nc.dram_tensor("qT", [D, NQ], F32R, kind="ExternalInput")
    kT_d = nc.dram_tensor("kT", [D, NK], F32R, kind="ExternalInput")
    vT_d = nc.dram_tensor("vT", [D, NK], F32R, kind="ExternalInput")
    w_d = {
        "wq": nc.dram_tensor("wqT", [D, D], F32R, kind="ExternalInput"),
        "wk": nc.dram_tensor("wkT", [D, D], F32R, kind="ExternalInput"),
        "wv": nc.dram_tensor("wvT", [D, D], F32R, kind="ExternalInput"),
    }
    out_d = nc.dram_tensor("out", [NQ, D], F32, kind="ExternalOutput")

    with tile.TileContext(nc) as tc:
        with tc.tile_pool(name="persist", bufs=1) as pp:
            ident_b = pp.tile([P, P], BF16, tag="ident_b")
            make_identity(nc, ident_b[:])

            # projected operands, persistent across phase C
            qp = [pp.tile([P, NQ], F32R, tag=f"qp{eb}", name=f"qp{eb}") for eb in range(NDB)]
            kp = [pp.tile([P, NK], F32R, tag=f"kp{eb}", name=f"kp{eb}") for eb in range(NDB)]
            vp = [pp.tile([P, D], BF16, tag=f"vp{jb}", name=f"vp{jb}") for jb in range(NJB)]

            # ---------------- Phase B: projections (PE order: k', v', q')
            with (
                tc.tile_pool(name="wp", bufs=1) as wp,
                tc.tile_pool(name="xp", bufs=2) as xp,
                tc.tile_pool(name="psP", bufs=4, space="PSUM") as psP,
            ):
                # weight tiles [d_local, db, e]; lhsT slices are [128, 128]
                wt = {}
                for wname in ("wk", "wv", "wq"):
                    t = wp.tile([P, NDB, D], F32R, tag=f"wt_{wname}", name=f"wt_{wname}")
                    nc.sync.dma_start(t[:], w_d[wname].rearrange("(a p) e -> p a e", p=P))
                    wt[wname] = t

                def load_x(xd, tag):
                    xt = xp.tile([P, NDB, NK], F32R, tag="xt", name=tag)
                    xre = xd.rearrange("(a p) n -> p a n", p=P)
                    for c in range(NJC):
                        sl = slice(c * JC, (c + 1) * JC)
                        nc.sync.dma_start(xt[:, :, sl], xre[:, :, sl])
                    return xt

                # k' projection: k'T[e, j] = sum_d WkT[d, e] kT[d, j]
                kt = load_x(kT_d, "kt")
                for c in range(NJC):
                    sl = slice(c * JC, (c + 1) * JC)
                    for eb in range(NDB):
                        pm = psP.tile([P, JC], F32, tag="pm")
                        for db in range(NDB):
                            nc.tensor.matmul(
                                pm[:],
                                wt["wk"][:, db, eb * P : (eb + 1) * P],
                                kt[:, db, sl],
                                start=(db == 0),
                                stop=(db == NDB - 1),
                            )
                        nc.any.tensor_copy(kp[eb][:, sl], pm[:])

                # v' projection: v'[j, d'] = sum_d vT[d, j] WvT[d, d']
                vt = load_x(vT_d, "vt")
                for jb in range(NJB):
                    pm = psP.tile([P, D], F32, tag="pm")
                    for db in range(NDB):
                        nc.tensor.matmul(
                            pm[:],
                            vt[:, db, jb * P : (jb + 1) * P],
                            wt["wv"][:, db, :],
                            start=(db == 0),
                            stop=(db == NDB - 1),
                        )
                    nc.any.tensor_copy(vp[jb][:], pm[:])

                # q' projection
                qt = load_x(qT_d, "qt")
                for c in range(NJC):
                    sl = slice(c * JC, (c + 1) * JC)
                    for eb in range(NDB):
                        pm = psP.tile([P, JC], F32, tag="pm")
                        for db in range(NDB):
                            nc.tensor.matmul(
                                pm[:],
                                wt["wq"][:, db, eb * P : (eb + 1) * P],
                                qt[:, db, sl],
                                start=(db == 0),
                                stop=(db == NDB - 1),
                            )
                        nc.any.tensor_copy(qp[eb][:, sl], pm[:])

            # ---------------- Phase C: attention, pipelined over query blocks
            with (
                tc.tile_pool(name="cs", bufs=2) as cs,
                tc.tile_pool(name="stat", bufs=2) as st,
                tc.tile_pool(name="psS", bufs=4, space="PSUM") as psS,
                tc.tile_pool(name="psT", bufs=2, space="PSUM") as psT,
                tc.tile_pool(name="psO", bufs=2, space="PSUM") as psO,
            ):
                def emit_scores(ib):
                    isl = slice(ib * P, (ib + 1) * P)
                    chunks = []
                    for c in range(NJC):
                        jsl = slice(c * JC, (c + 1) * JC)
                        sc_ = psS.tile([P, JC], F32, tag="sc")
                        for eb in range(NDB):
                            nc.tensor.matmul(
                                sc_[:],
                                qp[eb][:, isl],
                                kp[eb][:, jsl],
                                start=(eb == 0),
                                stop=(eb == NDB - 1),
                            )
                        chunks.append(sc_)
                    return chunks

                sch = emit_scores(0)
                for ib in range(NIB):
                    cur = sch
                    # --- softmax stats (DVE) + exp (ACT) for ib
                    nmax = []
                    for c in range(NJC):
                        nm = st.tile([P, 1], F32, tag=f"nm{c}", name=f"nm{c}")
                        nc.vector.reduce_max(nm[:], cur[c][:], axis=AX, negate=True)
                        nmax.append(nm)
                    nm01 = st.tile([P, 1], F32, tag="nm01")
                    nc.vector.tensor_tensor(nm01[:], nmax[0][:], nmax[1][:], op=ALU.min)
                    nm23 = st.tile([P, 1], F32, tag="nm23")
                    nc.vector.tensor_tensor(nm23[:], nmax[2][:], nmax[3][:], op=ALU.min)
                    nmall = st.tile([P, 1], F32, tag="nmall")
                    nc.vector.tensor_tensor(nmall[:], nm01[:], nm23[:], op=ALU.min)

                    w_sb = cs.tile([P, NK], BF16, tag="w")
                    dcs = []
                    for c in range(NJC):
                        dc = st.tile([P, 1], F32, tag=f"dc{c}", name=f"dc{c}")
                        nc.scalar.activation(
                            w_sb[:, c * JC : (c + 1) * JC],
                            cur[c][:],
                            EXP,
                            bias=nmall[:],
                            scale=1.0,
                            accum_out=dc[:],
                        )
                        dcs.append(dc)
                    d01 = st.tile([P, 1], F32, tag="d01")
                    nc.vector.tensor_tensor(d01[:], dcs[0][:], dcs[1][:], op=ALU.add)
                    d23 = st.tile([P, 1], F32, tag="d23")
                    nc.vector.tensor_tensor(d23[:], dcs[2][:], dcs[3][:], op=ALU.add)
                    den = st.tile([P, 1], F32, tag="den")
                    nc.vector.tensor_tensor(den[:], d01[:], d23[:], op=ALU.add)
                    rinv = st.tile([P, 1], F32, tag="rinv")
                    nc.vector.reciprocal(rinv[:], den[:])

                    # --- next block's scores keep the PE busy during exp(ib)
                    if ib + 1 < NIB:
                        sch = emit_scores(ib + 1)

                    # --- transpose exp weights (bf16, 1 cyc/col); bf16 PSUM
                    # tiles pack 8 transposes per bank and give 2x DVE copies
                    wT = cs.tile([P, NK], BF16, tag="wT")
                    for g in range(2):
                        pt = psT.tile([P, 8 * P], BF16, tag="pt")
                        for jj in range(8):
                            js = 8 * g + jj
                            nc.tensor.transpose(
                                pt[:, jj * P : (jj + 1) * P],
                                w_sb[:, js * P : (js + 1) * P],
                                ident_b[:],
                            )
                        nc.any.tensor_copy(wT[:, g * 8 * P : (g + 1) * 8 * P], pt[:])

                    # --- output matmul
                    po = psO.tile([P, D], F32, tag="po")
                    for js in range(NJB):
                        nc.tensor.matmul(
                            po[:],
                            wT[:, js * P : (js + 1) * P],
                            vp[js][:],
                            start=(js == 0),
                            stop=(js == NJB - 1),
                        )
                    ob = cs.tile([P, D], F32, tag="ob")
                    nc.vector.tensor_scalar_mul(ob[:], po[:], rinv[:])
                    nc.sync.dma_start(out_d[ib * P : (ib + 1) * P, :], ob[:])

    nc.compile()
    return nc


def _get_nc():
    if "nc" not in _CACHE:
        _CACHE["nc"] = _build()
    return _CACHE["nc"]


def kernel(query, key, value, Wq, Wk, Wv, _trace=False):
    query = np.asarray(query, dtype=np.float32)
    key = np.asarray(key, dtype=np.float32)
    value = np.asarray(value, dtype=np.float32)
    qT = np.ascontiguousarray(query.transpose(0, 2, 1))
    kT = np.ascontiguousarray(key.transpose(0, 2, 1))
    vT = np.ascontiguousarray(value.transpose(0, 2, 1))
    wqT = np.ascontiguousarray(np.asarray(Wq, dtype=np.float32).T)
    wkT = np.ascontiguousarray(np.asarray(Wk, dtype=np.float32).T)
    wvT = np.ascontiguousarray(np.asarray(Wv, dtype=np.float32).T)

    nc = _get_nc()
    in_maps = [
        {
            "qT": qT[b],
            "kT": kT[b],
            "vT": vT[b],
            "wqT": wqT,
            "wkT": wkT,
            "wvT": wvT,
        }
        for b in range(B)
    ]
    res = run_bass_kernel_spmd(nc, in_maps, list(range(B)), trace=_trace)
    out = np.stack([res.results[b]["out"] for b in range(B)]).astype(np.float32)
    if _trace:
        _CACHE["last_result"] = res
    return out


# revision 5
# speedup vs baseline: 2.0975x; 1.0165x over previous
"""CrossAttention TRN2 Bass kernel.

Problem: out[b] = softmax((q[b] @ Wq.T) @ (k[b] @ Wk.T).T) @ (v[b] @ Wv.T)
  q/k/v: [8, 2048, 512] f32, Wq/Wk/Wv: [512, 512] f32.

Sharding: data-parallel over batch -- core b computes batch b entirely.

Key optimizations vs the reference structure:
  * All operand transposes happen on the HOST (numpy) -- the device receives
    qT/kT/vT [D, N] and WqT/WkT/WvT [D, D], so the PE spends zero cycles
    transposing inputs.
  * Every projection / score matmul runs in f32r (fp32 bits, PE rounds
    operands to ~11-bit mantissa, 1 cycle/col vs fp32's 4).  Simulated
    end-to-end rel-err of this scheme is ~1.5e-2 against the 2e-2 gate.
  * Softmax weights are written as bf16 by the ACT exp, transposed on the PE
    at 1 cycle/col (vs 2 for fp32), and the output matmul runs bf16xbf16.
  * Phase C is software-pipelined: scores(ib+1) is issued to the PE between
    exp(ib) (ACT) and the weight-transposes(ib), so the PE never waits on
    the softmax statistics chain.

Per-core PE budget @2.4GHz: k'/q' proj 66k cyc, v' proj 33k, scores 131k,
w transposes 33k, output 131k  ->  ~165us + DMA lead-in.
"""
import sys

if "/opt/trn_rl_repo" not in sys.path:
    sys.path.insert(0, "/opt/trn_rl_repo")

import numpy as np

import concourse.bacc as bacc
import concourse.mybir as mybir
import concourse.tile as tile
from concourse.bass_utils import run_bass_kernel_spmd
from concourse.masks import make_identity

F32 = mybir.dt.float32
F32R = mybir.dt.float32r
BF16 = mybir.dt.bfloat16
AX = mybir.AxisListType.X
ALU = mybir.AluOpType
EXP = mybir.ActivationFunctionType.Exp

B, NQ, NK, D = 8, 2048, 2048, 512
P = 128
NDB = D // P    # feature blocks (4)
NIB = NQ // P   # query row blocks (16)
NJB = NK // P   # key row blocks (16)
JC = 512        # scores j-chunk width (one fp32 PSUM bank)
NJC = NK // JC  # 4

_CACHE = {}


def _build():
    nc = bacc.Bacc("TRN2", target_bir_lowering=False)
    qT_d = nc.dram_tensor("qT", [D, NQ], F32R, kind="ExternalInput")
    kT_d = nc.dram_tensor("kT", [D, NK], F32R, kind="ExternalInput")
    vT_d = nc.dram_tensor("vT", [D, NK], F32R, kind="ExternalInput")
    w_d = {
        "wq": nc.dram_tensor("wqT", [D, D], F32R, kind="ExternalInput"),
        "wk": nc.dram_tensor("wkT", [D, D], F32R, kind="ExternalInput"),
        "wv": nc.dram_tensor("wvT", [D, D], F32R, kind="ExternalInput"),
    }
    out_d = nc.dram_tensor("out", [NQ, D], F32, kind="ExternalOutput")

    with tile.TileContext(nc) as tc:
        with tc.tile_pool(name="persist", bufs=1) as pp:
            ident_b = pp.tile([P, P], BF16, tag="ident_b")
            make_identity(nc, ident_b[:])

            # projected operands, persistent across phase C
            qp = [pp.tile([P, NQ], F32R, tag=f"qp{eb}", name=f"qp{eb}") for eb in range(NDB)]
            kp = [pp.tile([P, NK], F32R, tag=f"kp{eb}", name=f"kp{eb}") for eb in range(NDB)]
            vp = [pp.tile([P, D], BF16, tag=f"vp{jb}", name=f"vp{jb}") for jb in range(NJB)]

            # ---------------- Phase B: projections (PE order: k', q', v')
            with (
                tc.tile_pool(name="wp", bufs=1) as wp,
                tc.tile_pool(name="xp", bufs=1) as xp,
                tc.tile_pool(name="psP", bufs=4, space="PSUM") as psP,
            ):
                # All input DMAs in consume-order.  k/q path on the SP (sync)
                # hwdge queue, v path + W_v on the ACT (scalar) queue so the
                # two transfer streams overlap; per-chunk tiles give
                # chunk-granular dependencies so the PE starts after the
                # first ~2MB rather than after the full 15MB.
                def wtile(wname, eng):
                    t = wp.tile([P, NDB, D], F32R, tag=f"wt_{wname}", name=f"wt_{wname}")
                    eng.dma_start(t[:], w_d[wname].rearrange("(a p) e -> p a e", p=P))
                    return t

                def load_x(xd, tag, eng):
                    xre = xd.rearrange("(a p) n -> p a n", p=P)
                    chunks = []
                    for c in range(NJC):
                        sl = slice(c * JC, (c + 1) * JC)
                        xt = xp.tile([P, NDB, JC], F32R, tag=f"{tag}{c}", name=f"{tag}{c}")
                        eng.dma_start(xt[:], xre[:, :, sl])
                        chunks.append(xt)
                    return chunks

                wt = {}
                wt["wk"] = wtile("wk", nc.sync)
                kt = load_x(kT_d, "kt", nc.sync)
                wt["wq"] = wtile("wq", nc.sync)
                qt = load_x(qT_d, "qt", nc.sync)
                wt["wv"] = wtile("wv", nc.scalar)
                vt = load_x(vT_d, "vt", nc.scalar)

                # k' projection: k'T[e, j] = sum_d WkT[d, e] kT[d, j]
                for c in range(NJC):
                    sl = slice(c * JC, (c + 1) * JC)
                    for eb in range(NDB):
                        pm = psP.tile([P, JC], F32, tag="pm")
                        for db in range(NDB):
                            nc.tensor.matmul(
                                pm[:],
                                wt["wk"][:, db, eb * P : (eb + 1) * P],
                                kt[c][:, db, :],
                                start=(db == 0),
                                stop=(db == NDB - 1),
                            )
                        nc.any.tensor_copy(kp[eb][:, sl], pm[:])

                # q' projection
                for c in range(NJC):
                    sl = slice(c * JC, (c + 1) * JC)
                    for eb in range(NDB):
                        pm = psP.tile([P, JC], F32, tag="pm")
                        for db in range(NDB):
                            nc.tensor.matmul(
                                pm[:],
                                wt["wq"][:, db, eb * P : (eb + 1) * P],
                                qt[c][:, db, :],
                                start=(db == 0),
                                stop=(db == NDB - 1),
                            )
                        nc.any.tensor_copy(qp[eb][:, sl], pm[:])

                # v' projection: v'[j, d'] = sum_d vT[d, j] WvT[d, d']
                for jb in range(NJB):
                    pm = psP.tile([P, D], F32, tag="pm")
                    for db in range(NDB):
                        nc.tensor.matmul(
                            pm[:],
                            vt[jb // 4][:, db, (jb % 4) * P : (jb % 4 + 1) * P],
                            wt["wv"][:, db, :],
                            start=(db == 0),
                            stop=(db == NDB - 1),
                        )
                    nc.any.tensor_copy(vp[jb][:], pm[:])

            # ---------------- Phase C: attention, pipelined over query blocks
            with (
                tc.tile_pool(name="cs", bufs=2) as cs,
                tc.tile_pool(name="stat", bufs=2) as st,
                tc.tile_pool(name="psS", bufs=5, space="PSUM") as psS,
                tc.tile_pool(name="psT", bufs=2, space="PSUM") as psT,
                tc.tile_pool(name="psO", bufs=1, space="PSUM") as psO,
            ):
                def emit_scores(ib):
                    isl = slice(ib * P, (ib + 1) * P)
                    chunks = []
                    for c in range(NJC):
                        jsl = slice(c * JC, (c + 1) * JC)
                        sc_ = psS.tile([P, JC], F32, tag="sc")
                        for eb in range(NDB):
                            nc.tensor.matmul(
                                sc_[:],
                                qp[eb][:, isl],
                                kp[eb][:, jsl],
                                start=(eb == 0),
                                stop=(eb == NDB - 1),
                            )
                        chunks.append(sc_)
                    return chunks

                sch = emit_scores(0)
                for ib in range(NIB):
                    cur = sch
                    # --- softmax stats (DVE) + exp (ACT) for ib
                    nmax = []
                    for c in range(NJC):
                        nm = st.tile([P, 1], F32, tag=f"nm{c}", name=f"nm{c}")
                        nc.vector.reduce_max(nm[:], cur[c][:], axis=AX, negate=True)
                        nmax.append(nm)
                    nm01 = st.tile([P, 1], F32, tag="nm01")
                    nc.vector.tensor_tensor(nm01[:], nmax[0][:], nmax[1][:], op=ALU.min)
                    nm23 = st.tile([P, 1], F32, tag="nm23")
                    nc.vector.tensor_tensor(nm23[:], nmax[2][:], nmax[3][:], op=ALU.min)
                    nmall = st.tile([P, 1], F32, tag="nmall")
                    nc.vector.tensor_tensor(nmall[:], nm01[:], nm23[:], op=ALU.min)

                    w_sb = cs.tile([P, NK], BF16, tag="w")
                    dcs = []
                    for c in range(NJC):
                        dc = st.tile([P, 1], F32, tag=f"dc{c}", name=f"dc{c}")
                        nc.scalar.activation(
                            w_sb[:, c * JC : (c + 1) * JC],
                            cur[c][:],
                            EXP,
                            bias=nmall[:],
                            scale=1.0,
                            accum_out=dc[:],
                        )
                        dcs.append(dc)
                    d01 = st.tile([P, 1], F32, tag="d01")
                    nc.vector.tensor_tensor(d01[:], dcs[0][:], dcs[1][:], op=ALU.add)
                    d23 = st.tile([P, 1], F32, tag="d23")
                    nc.vector.tensor_tensor(d23[:], dcs[2][:], dcs[3][:], op=ALU.add)
                    den = st.tile([P, 1], F32, tag="den")
                    nc.vector.tensor_tensor(den[:], d01[:], d23[:], op=ALU.add)
                    rinv = st.tile([P, 1], F32, tag="rinv")
                    nc.vector.reciprocal(rinv[:], den[:])

                    # --- next block's scores keep the PE busy during exp(ib)
                    if ib + 1 < NIB:
                        sch = emit_scores(ib + 1)

                    # --- transpose exp weights (bf16, 1 cyc/col); bf16 PSUM
                    # tiles pack 8 transposes per bank and give 2x DVE copies
                    wT = cs.tile([P, NK], BF16, tag="wT")
                    for g in range(2):
                        pt = psT.tile([P, 8 * P], BF16, tag="pt")
                        for jj in range(8):
                            js = 8 * g + jj
                            nc.tensor.transpose(
                                pt[:, jj * P : (jj + 1) * P],
                                w_sb[:, js * P : (js + 1) * P],
                                ident_b[:],
                            )
                        nc.vector.tensor_copy(wT[:, g * 8 * P : (g + 1) * 8 * P], pt[:])

                    # --- output matmul
                    po = psO.tile([P, D], F32, tag="po")
                    for js in range(NJB):
                        nc.tensor.matmul(
                            po[:],
                            wT[:, js * P : (js + 1) * P],
                            vp[js][:],
                            start=(js == 0),
                            stop=(js == NJB - 1),
                        )
                    ob = cs.tile([P, D], F32, tag="ob")
                    nc.vector.tensor_scalar_mul(ob[:], po[:], rinv[:])
                    nc.sync.dma_start(out_d[ib * P : (ib + 1) * P, :], ob[:])

    nc.compile()
    return nc


def _get_nc():
    if "nc" not in _CACHE:
        _CACHE["nc"] = _build()
    return _CACHE["nc"]


def kernel(query, key, value, Wq, Wk, Wv, _trace=False):
    query = np.asarray(query, dtype=np.float32)
    key = np.asarray(key, dtype=np.float32)
    value = np.asarray(value, dtype=np.float32)
    qT = np.ascontiguousarray(query.transpose(0, 2, 1))
    kT = np.ascontiguousarray(key.transpose(0, 2, 1))
    vT = np.ascontiguousarray(value.transpose(0, 2, 1))
    wqT = np.ascontiguousarray(np.asarray(Wq, dtype=np.float32).T)
    wkT = np.ascontiguousarray(np.asarray(Wk, dtype=np.float32).T)
    wvT = np.ascontiguousarray(np.asarray(Wv, dtype=np.float32).T)

    nc = _get_nc()
    in_maps = [
        {
            "qT": qT[b],
            "kT": kT[b],
            "vT": vT[b],
            "wqT": wqT,
            "wkT": wkT,
            "wvT": wvT,
        }
        for b in range(B)
    ]
    res = run_bass_kernel_spmd(nc, in_maps, list(range(B)), trace=_trace)
    out = np.stack([res.results[b]["out"] for b in range(B)]).astype(np.float32)
    if _trace:
        _CACHE["last_result"] = res
    return out


# revision 10
# speedup vs baseline: 2.2463x; 1.0710x over previous
"""CrossAttention TRN2 Bass kernel.

Problem: out[b] = softmax((q[b] @ Wq.T) @ (k[b] @ Wk.T).T) @ (v[b] @ Wv.T)
  q/k/v: [8, 2048, 512] f32, Wq/Wk/Wv: [512, 512] f32.

Sharding: data-parallel over batch -- core b computes batch b entirely.

Key optimizations vs the reference structure:
  * All operand transposes happen on the HOST (numpy) -- the device receives
    qT/kT/vT [D, N] and WqT/WkT/WvT [D, D], so the PE spends zero cycles
    transposing inputs.
  * Every projection / score matmul runs in f32r (fp32 bits, PE rounds
    operands to ~11-bit mantissa, 1 cycle/col vs fp32's 4).  Simulated
    end-to-end rel-err of this scheme is ~1.5e-2 against the 2e-2 gate.
  * Softmax weights are written as bf16 by the ACT exp, transposed on the PE
    at 1 cycle/col (vs 2 for fp32), and the output matmul runs bf16xbf16.
  * Phase C is software-pipelined: scores(ib+1) is issued to the PE between
    exp(ib) (ACT) and the weight-transposes(ib), so the PE never waits on
    the softmax statistics chain.

Per-core PE budget @2.4GHz: k'/q' proj 66k cyc, v' proj 33k, scores 131k,
w transposes 33k, output 131k  ->  ~165us + DMA lead-in.
"""
import sys

if "/opt/trn_rl_repo" not in sys.path:
    sys.path.insert(0, "/opt/trn_rl_repo")

import numpy as np

import concourse.bacc as bacc
import concourse.mybir as mybir
import concourse.tile as tile
from concourse.bass_utils import run_bass_kernel_spmd
from concourse.masks import make_identity

F32 = mybir.dt.float32
F32R = mybir.dt.float32r
BF16 = mybir.dt.bfloat16
AX = mybir.AxisListType.X
ALU = mybir.AluOpType
EXP = mybir.ActivationFunctionType.Exp

B, NQ, NK, D = 8, 2048, 2048, 512
P = 128
NDB = D // P    # feature blocks (4)
NIB = NQ // P   # query row blocks (16)
NJB = NK // P   # key row blocks (16)
JC = 512        # scores j-chunk width (one fp32 PSUM bank)
NJC = NK // JC  # 4

_CACHE = {}


def _build():
    nc = bacc.Bacc("TRN2", target_bir_lowering=False)
    qT_d = nc.dram_tensor("qT", [D, NQ], F32R, kind="ExternalInput")
    kT_d = nc.dram_tensor("kT", [D, NK], F32R, kind="ExternalInput")
    vT_d = nc.dram_tensor("vT", [D, NK], F32R, kind="ExternalInput")
    w_d = {
        # wq/wk NATIVE [e, d] (feed MT = Wk^T Wq contraction over e);
        # wv transposed [d, d'] (rhs of the v' projection)
        "wq": nc.dram_tensor("wqN", [D, D], F32R, kind="ExternalInput"),
        "wk": nc.dram_tensor("wkN", [D, D], F32R, kind="ExternalInput"),
        "wv": nc.dram_tensor("wvT", [D, D], F32R, kind="ExternalInput"),
    }
    out_d = nc.dram_tensor("out", [NQ, D], F32, kind="ExternalOutput")

    with tile.TileContext(nc) as tc:
        with tc.tile_pool(name="persist", bufs=1) as pp:
            ident_b = pp.tile([P, P], BF16, tag="ident_b")
            make_identity(nc, ident_b[:])

            # persistent across phase C: raw qT chunks (scores lhsT), the
            # folded Mk = (Wq^T Wk) k^T (scores rhs), and v'
            qt = [pp.tile([P, NDB, JC], F32R, tag=f"qt{c}", name=f"qt{c}") for c in range(NJC)]
            mk = [pp.tile([P, NK], F32R, tag=f"mk{db}", name=f"mk{db}") for db in range(NDB)]
            vp = [pp.tile([P, D], BF16, tag=f"vp{jb}", name=f"vp{jb}") for jb in range(NJB)]

            # ---------------- Phase B (PE order: MT, Mk, v')
            # scores = q' k'^T = q (Wq^T Wk) k^T: fold the two projection
            # weights into M once (tiny), apply M to k^T only, and feed raw
            # qT as the scores stationary -- deletes the whole q' projection.
            with (
                tc.tile_pool(name="wp", bufs=1) as wp,
                tc.tile_pool(name="xp", bufs=1) as xp,
                tc.tile_pool(name="psP", bufs=4, space="PSUM") as psP,
            ):
                # Critical path on the SP (sync) hwdge queue: Wk, Wq, kT.
                # Everything else (Wv, vT, qT) on the ACT (scalar) queue.
                def wtile(wname, eng):
                    t = wp.tile([P, NDB, D], F32R, tag=f"wt_{wname}", name=f"wt_{wname}")
                    eng.dma_start(t[:], w_d[wname].rearrange("(a p) e -> p a e", p=P))
                    return t

                def load_x(xd, tiles, eng):
                    xre = xd.rearrange("(a p) n -> p a n", p=P)
                    for c in range(NJC):
                        sl = slice(c * JC, (c + 1) * JC)
                        eng.dma_start(tiles[c][:], xre[:, :, sl])
                    return tiles

                wt = {}
                wt["wk"] = wtile("wk", nc.sync)
                wt["wq"] = wtile("wq", nc.sync)
                kt = load_x(kT_d, [xp.tile([P, NDB, JC], F32R, tag=f"kt{c}", name=f"kt{c}") for c in range(NJC)], nc.sync)
                wt["wv"] = wtile("wv", nc.scalar)
                vt = load_x(vT_d, [xp.tile([P, NDB, JC], F32R, tag=f"vt{c}", name=f"vt{c}") for c in range(NJC)], nc.scalar)
                load_x(qT_d, qt, nc.scalar)

                # MT[d2, d1] = sum_e Wk[e, d2] Wq[e, d1]  (16 matmuls)
                mtt = [wp.tile([P, D], F32R, tag=f"mtt{b2}", name=f"mtt{b2}") for b2 in range(NDB)]
                for b2 in range(NDB):
                    pm = psP.tile([P, D], F32, tag="pm")
                    for a in range(NDB):
                        nc.tensor.matmul(
                            pm[:],
                            wt["wk"][:, a, b2 * P : (b2 + 1) * P],
                            wt["wq"][:, a, :],
                            start=(a == 0),
                            stop=(a == NDB - 1),
                        )
                    nc.any.tensor_copy(mtt[b2][:], pm[:])

                # Mk[d1, j] = sum_d2 MT[d2, d1] kT[d2, j]  (64 matmuls)
                for c in range(NJC):
                    sl = slice(c * JC, (c + 1) * JC)
                    for b1 in range(NDB):
                        pm = psP.tile([P, JC], F32, tag="pm")
                        for b2 in range(NDB):
                            nc.tensor.matmul(
                                pm[:],
                                mtt[b2][:, b1 * P : (b1 + 1) * P],
                                kt[c][:, b2, :],
                                start=(b2 == 0),
                                stop=(b2 == NDB - 1),
                            )
                        nc.any.tensor_copy(mk[b1][:, sl], pm[:])

                # v' projection: v'[j, d'] = sum_d vT[d, j] WvT[d, d']
                for jb in range(NJB):
                    pm = psP.tile([P, D], F32, tag="pm")
                    for db in range(NDB):
                        nc.tensor.matmul(
                            pm[:],
                            vt[jb // 4][:, db, (jb % 4) * P : (jb % 4 + 1) * P],
                            wt["wv"][:, db, :],
                            start=(db == 0),
                            stop=(db == NDB - 1),
                        )
                    nc.any.tensor_copy(vp[jb][:], pm[:])

            # ---------------- Phase C: attention, pipelined over query blocks
            with (
                tc.tile_pool(name="cs", bufs=2) as cs,
                tc.tile_pool(name="stat", bufs=2) as st,
                tc.tile_pool(name="psS", bufs=5, space="PSUM") as psS,
                tc.tile_pool(name="psT", bufs=2, space="PSUM") as psT,
                tc.tile_pool(name="psO", bufs=1, space="PSUM") as psO,
            ):
                def emit_scores(ib):
                    qtile = qt[ib // 4]
                    io = (ib % 4) * P
                    chunks = []
                    for c in range(NJC):
                        jsl = slice(c * JC, (c + 1) * JC)
                        sc_ = psS.tile([P, JC], F32, tag="sc")
                        for b1 in range(NDB):
                            nc.tensor.matmul(
                                sc_[:],
                                qtile[:, b1, io : io + P],
                                mk[b1][:, jsl],
                                start=(b1 == 0),
                                stop=(b1 == NDB - 1),
                            )
                        chunks.append(sc_)
                    return chunks

                sch = emit_scores(0)
                for ib in range(NIB):
                    cur = sch
                    # --- softmax stats (DVE) + exp (ACT) for ib
                    nmax = []
                    for c in range(NJC):
                        nm = st.tile([P, 1], F32, tag=f"nm{c}", name=f"nm{c}")
                        nc.vector.reduce_max(nm[:], cur[c][:], axis=AX, negate=True)
                        nmax.append(nm)
                    nm01 = st.tile([P, 1], F32, tag="nm01")
                    nc.vector.tensor_tensor(nm01[:], nmax[0][:], nmax[1][:], op=ALU.min)
                    nm23 = st.tile([P, 1], F32, tag="nm23")
                    nc.vector.tensor_tensor(nm23[:], nmax[2][:], nmax[3][:], op=ALU.min)
                    nmall = st.tile([P, 1], F32, tag="nmall")
                    nc.vector.tensor_tensor(nmall[:], nm01[:], nm23[:], op=ALU.min)

                    w_sb = cs.tile([P, NK], BF16, tag="w")
                    dcs = []
                    for c in range(NJC):
                        dc = st.tile([P, 1], F32, tag=f"dc{c}", name=f"dc{c}")
                        nc.scalar.activation(
                            w_sb[:, c * JC : (c + 1) * JC],
                            cur[c][:],
                            EXP,
                            bias=nmall[:],
                            scale=1.0,
                            accum_out=dc[:],
                        )
                        dcs.append(dc)
                    d01 = st.tile([P, 1], F32, tag="d01")
                    nc.vector.tensor_tensor(d01[:], dcs[0][:], dcs[1][:], op=ALU.add)
                    d23 = st.tile([P, 1], F32, tag="d23")
                    nc.vector.tensor_tensor(d23[:], dcs[2][:], dcs[3][:], op=ALU.add)
                    den = st.tile([P, 1], F32, tag="den")
                    nc.vector.tensor_tensor(den[:], d01[:], d23[:], op=ALU.add)
                    rinv = st.tile([P, 1], F32, tag="rinv")
                    nc.vector.reciprocal(rinv[:], den[:])

                    # --- next block's scores keep the PE busy during exp(ib)
                    if ib + 1 < NIB:
                        sch = emit_scores(ib + 1)

                    # --- transpose exp weights (bf16, 1 cyc/col); bf16 PSUM
                    # tiles pack 8 transposes per bank and give 2x DVE copies
                    wT = cs.tile([P, NK], BF16, tag="wT")
                    for g in range(2):
                        pt = psT.tile([P, 8 * P], BF16, tag="pt")
                        for jj in range(8):
                            js = 8 * g + jj
                            nc.tensor.transpose(
                                pt[:, jj * P : (jj + 1) * P],
                                w_sb[:, js * P : (js + 1) * P],
                                ident_b[:],
                            )
                        nc.vector.tensor_copy(wT[:, g * 8 * P : (g + 1) * 8 * P], pt[:])

                    # --- output matmul
                    po = psO.tile([P, D], F32, tag="po")
                    for js in range(NJB):
                        nc.tensor.matmul(
                            po[:],
                            wT[:, js * P : (js + 1) * P],
                            vp[js][:],
                            start=(js == 0),
                            stop=(js == NJB - 1),
                        )
                    ob = cs.tile([P, D], F32, tag="ob")
                    nc.vector.tensor_scalar_mul(ob[:], po[:], rinv[:])
                    nc.sync.dma_start(out_d[ib * P : (ib + 1) * P, :], ob[:])

    nc.compile()
    return nc


def _get_nc():
    if "nc" not in _CACHE:
        _CACHE["nc"] = _build()
    return _CACHE["nc"]


def kernel(query, key, value, Wq, Wk, Wv, _trace=False):
    query = np.asarray(query, dtype=np.float32)
    key = np.asarray(key, dtype=np.float32)
    value = np.asarray(value, dtype=np.float32)
    qT = np.ascontiguousarray(query.transpose(0, 2, 1))
    kT = np.ascontiguousarray(key.transpose(0, 2, 1))
    vT = np.ascontiguousarray(value.transpose(0, 2, 1))
    wqN = np.ascontiguousarray(np.asarray(Wq, dtype=np.float32))
    wkN = np.ascontiguousarray(np.asarray(Wk, dtype=np.float32))
    wvT = np.ascontiguousarray(np.asarray(Wv, dtype=np.float32).T)

    nc = _get_nc()
    in_maps = [
        {
            "qT": qT[b],
            "kT": kT[b],
            "vT": vT[b],
            "wqN": wqN,
            "wkN": wkN,
            "wvT": wvT,
        }
        for b in range(B)
    ]
    res = run_bass_kernel_spmd(nc, in_maps, list(range(B)), trace=_trace)
    out = np.stack([res.results[b]["out"] for b in range(B)]).astype(np.float32)
    if _trace:
        _CACHE["last_result"] = res
    return out
